# revision 7
# baseline (speedup 1.0000x reference)
"""LIF spike-train scan (nn_LIFSpike) on 8 TRN2 NeuronCores.

Reference semantics (fp32, bit-exact):
    u_t = TAU * u_{t-1} * (1 - o_{t-1}) + x_t ;  o_t = (u_t > VTH)
with u_{-1} = o_{-1} = 0, scanned over the trailing time dim (T=50).

Sharding: pure data parallel — the 16*64*32*32 = 1,048,576 spatial elements
are split evenly across 8 cores (131,072 each); the time scan runs on-chip.

On-chip layout per core: tiles of [128 partitions, F spatial, 50 time], time
scanned sequentially with all-spatial-parallel vector ops.  Per step:
    g   = u * [u <= VTH]          (scalar_tensor_tensor / fused)
    u'  = TAU * g + x_t           (scalar_tensor_tensor / fused)
    o_t = [u' > VTH]              (tensor_scalar is_gt)
which reproduces the reference rounding exactly: round(TAU*u) then *{0,1}
then round(+x) == round(TAU*(u*{0,1})) + x for each branch.
"""

import os
import numpy as np

import concourse.bass as bass
import concourse.bacc as bacc
import concourse.tile as tile
from concourse import mybir
from concourse.bass_utils import run_bass_kernel_spmd

TAU = 0.3
VTH = 0.3

T = 50
S_FULL = 16 * 64 * 32 * 32          # 1,048,576 spatial elements
N_CORES = 8
S_CORE = S_FULL // N_CORES          # 131,072
P = 128                             # SBUF partitions
F = 128                             # spatial elements per partition per tile
NB = S_CORE // (P * F)              # tiles per core

USE_FUSED = os.environ.get("LIF_FUSED", "1") == "1"

# results of the last run (for test.py to inspect trace/exec time)
LAST_RESULTS = None

_FUSED_OP = None


def _get_fused_op():
    """Register the fused gated-leak op: out = select(VTH >= u, u, 0)*TAU + x.

    One DVE instruction per scan step instead of two scalar_tensor_tensor
    passes.  Registered at runtime into concourse.dve_ops' module-level
    registry (OPS / CUSTOM_DVE_SPECS / opcode map), which is all the
    table-gen path reads."""
    global _FUSED_OP
    if _FUSED_OP is not None:
        return _FUSED_OP
    import concourse.dve_ops as dve_ops
    from concourse.dve_spec import Spec, Src0, Src1, C0, C1, Zero, select, lower
    from concourse.dve_uop import DveOpSpec

    name = "LIF_GATED_LEAK_ANT"
    spec = Spec(
        body=select(C0 >= Src0, Src0, Zero) * C1 + Src1,
        reference=lambda in0, in1, s0, s1, imm2: (
            np.where(s0 >= in0, in0, np.float32(0.0)).astype(np.float32) * np.float32(s1)
        ).astype(np.float32)
        + in1,
    )
    existing = {op.name for op in dve_ops.OPS}
    if name not in existing:
        row = dve_ops._CUSTOM_DVE_ROW_BASE + len(dve_ops.OPS)
        assert row < 0x20, "custom-DVE opcode row overflow"
        # pin the sha to what lower() actually produces (self-consistent)
        shas = {}
        for ver in ("v3", "v4"):
            uops = lower(spec, ver=ver)
            shas[ver] = DveOpSpec(name=name, opcode=row, uops=uops, rd1_en=True).sha(ver)
        op = dve_ops.DveOp(name, spec, subdim=False, uops_sha=shas)
        dve_ops.OPS.append(op)
        dve_ops.CUSTOM_DVE_SPECS[name] = spec
        dve_ops._SUB_OPCODE_FOR_NAME[name] = row
        _FUSED_OP = op
    else:
        _FUSED_OP = next(op for op in dve_ops.OPS if op.name == name)
    return _FUSED_OP


def _build_program():
    f32 = mybir.dt.float32
    nc = bacc.Bacc("TRN2", target_bir_lowering=False, debug=False)

    x_d = nc.dram_tensor("x", [NB, P, F, T], f32, kind="ExternalInput").ap()
    o_d = nc.dram_tensor("o", [NB, P, F, T], f32, kind="ExternalOutput").ap()

    fused = _get_fused_op() if USE_FUSED else None

    with tile.TileContext(nc) as tc:
        with (
            tc.tile_pool(name="xp", bufs=3) as xp,
            tc.tile_pool(name="op", bufs=2) as op_,
            tc.tile_pool(name="up", bufs=2) as up,
            tc.tile_pool(name="gp", bufs=2) as gp,
        ):
            for b in range(NB):
                xt = xp.tile([P, F, T], f32)
                nc.gpsimd.dma_start(out=xt[:], in_=x_d[b])
                ot = op_.tile([P, F, T], f32)

                u = None
                for t in range(T):
                    u_new = up.tile([P, F], f32)
                    if t == 0:
                        # u_0 = x_0 (carry is zero)
                        nc.vector.tensor_copy(u_new[:], xt[:, :, t])
                    elif fused is not None:
                        nc.vector._custom_dve(
                            fused,
                            out=u_new[:],
                            in0=u[:],
                            in1=xt[:, :, t],
                            s0=VTH,
                            s1=TAU,
                        )
                    else:
                        g = gp.tile([P, F], f32)
                        nc.vector.scalar_tensor_tensor(
                            g[:], u[:], VTH, u[:],
                            mybir.AluOpType.is_le, mybir.AluOpType.mult,
                        )
                        nc.vector.scalar_tensor_tensor(
                            u_new[:], g[:], TAU, xt[:, :, t],
                            mybir.AluOpType.mult, mybir.AluOpType.add,
                        )
                    u = u_new
                    nc.vector.tensor_scalar(
                        ot[:, :, t], u[:], VTH, None, mybir.AluOpType.is_gt
                    )

                nc.gpsimd.dma_start(out=o_d[b], in_=ot[:])
    nc.compile()
    return nc


def _make_runner(nc):
    """Jitted 8-core runner over device-resident buffers (for benchmarking).

    Mirrors bass2jax.run_bass_via_pjrt's shard_map construction but without
    donation, so input buffers stay alive across repeated timed calls.  The
    kernel writes every output element, so the output-seed buffer contents
    are irrelevant."""
    import jax
    import jax.numpy as jnp
    from jax.sharding import Mesh, PartitionSpec, NamedSharding
    from jax.experimental.shard_map import shard_map
    from concourse import bass2jax, mybir as _mybir

    bass2jax.install_neuronx_cc_hook()

    in_names, out_names, out_avals = [], [], []
    for alloc in nc.m.functions[0].allocations:
        if not isinstance(alloc, mybir.MemoryLocationSet):
            continue
        name = alloc.memorylocations[0].name
        if alloc.kind == "ExternalInput":
            if nc.partition_id_tensor is None or name != nc.partition_id_tensor.name:
                in_names.append(name)
        elif alloc.kind == "ExternalOutput":
            out_names.append(name)
            out_avals.append(
                jax.core.ShapedArray(tuple(alloc.tensor_shape), _mybir.dt.np(alloc.dtype))
            )
    all_in = list(in_names) + list(out_names)
    if nc.partition_id_tensor is not None:
        all_in.append(nc.partition_id_tensor.name)

    def _body(*args):
        operands = list(args)
        if nc.partition_id_tensor is not None:
            operands.append(bass2jax.partition_id_tensor())
        return tuple(
            bass2jax._bass_exec_p.bind(
                *operands,
                out_avals=tuple(out_avals),
                in_names=tuple(all_in),
                out_names=tuple(out_names),
                lowering_input_output_aliases=(),
                sim_require_finite=True,
                sim_require_nnan=True,
                nc=nc,
            )
        )

    devices = jax.devices()[:N_CORES]
    mesh = Mesh(np.asarray(devices), ("core",))
    n_ops = len(in_names) + len(out_names)
    fn = jax.jit(
        shard_map(
            _body,
            mesh=mesh,
            in_specs=(PartitionSpec("core"),) * n_ops,
            out_specs=(PartitionSpec("core"),) * len(out_names),
            check_rep=False,
        ),
        keep_unused=True,
    )
    sh = NamedSharding(mesh, PartitionSpec("core"))
    return fn, sh, out_avals


def bench(x, iters=10):
    """Compile once, device_put inputs, time repeated executions."""
    import time as _time
    import jax

    x = np.ascontiguousarray(np.asarray(x, dtype=np.float32)).reshape(S_FULL, T)
    nc = _build_program()
    fn, sh, out_avals = _make_runner(nc)
    xg = x.reshape(N_CORES * NB, P, F, T)
    xdev = jax.device_put(xg, sh)
    zdev = jax.device_put(
        np.zeros((N_CORES * out_avals[0].shape[0], *out_avals[0].shape[1:]), np.float32), sh
    )
    # warmup + compile
    out = fn(xdev, zdev)
    jax.block_until_ready(out)
    times = []
    for _ in range(iters):
        t0 = _time.perf_counter()
        out = fn(xdev, zdev)
        jax.block_until_ready(out)
        times.append(_time.perf_counter() - t0)
    arr = np.asarray(out[0]).reshape(S_FULL, T)
    return times, arr


def kernel(x, ksi=None, trace=False):
    """Full-input entry: x [16,64,32,32,50] f32 -> spikes, same shape.
    (ksi is unused by the reference computation.)"""
    global LAST_RESULTS
    x = np.ascontiguousarray(np.asarray(x, dtype=np.float32))
    orig_shape = x.shape
    xf = x.reshape(S_FULL, T)

    nc = _build_program()

    in_maps = [
        {"x": xf[i * S_CORE:(i + 1) * S_CORE].reshape(NB, P, F, T)}
        for i in range(N_CORES)
    ]
    res = run_bass_kernel_spmd(nc, in_maps, list(range(N_CORES)), trace=trace)
    LAST_RESULTS = res

    out = np.empty((S_FULL, T), dtype=np.float32)
    for i in range(N_CORES):
        out[i * S_CORE:(i + 1) * S_CORE] = res.results[i]["o"].reshape(S_CORE, T)
    return out.reshape(orig_shape)
